# revision 8
# baseline (speedup 1.0000x reference)
"""Trainium2 Bass kernel for nn_ATTLayer (attention pooling).

Reference computation (full input [64, 512, 1024] fp32):
    wb    = attention_w + attention_b          # [1024, 256] (b broadcast over rows)
    u_t   = tanh(inputs @ wb)                  # [64, 512, 256]
    logit = u_t @ attention_u                  # [64, 512]
    w     = softmax(logit, axis=1)             # softmax over seq
    out   = sum_s w[:, s] * inputs[:, s, :]    # [64, 1024]

Sharding: data-parallel over batch — 8 batches per core on 8 NeuronCores, no
collectives. Tiny params (wb = W + b, u) are precomputed/replicated on host.

Single-upload design: x is cast to bf16 and uploaded ONLY in block-transposed
layout xt[b, k, h_local, s] (h on SBUF partitions), halving HBM traffic vs a
dual-layout upload. The weighted sum, which needs contraction over s, runs on
the DVE as a fused multiply+reduce along the free axis instead of on the PE
(which would need s on partitions). Per core, per batch b:

  1. DMA xt[b] (split across the SP and Activation HWDGE queues)
  2. PE GEMM1: psum[a_chunk, s] += wb[k, a_chunk].T @ xt[k]  (k = 8 h-chunks)
     -> tanh on ScalarE -> u_t^T bf16 [a_local, s] tiles
  3. PE logit row: psum[1, s] += u[a_chunk].T @ u_t^T[a_chunk]  (u stationary)
  4. ScalarE exp -> wt_row [1, 512] bf16 with fused accum_out = sum (fp32);
     no max-subtraction (|logit| <= ~20 keeps exp finite in fp32); the 1/sum
     normalization is applied to the final [128, 8] output block.
  5. DVE reciprocal -> rs; tiny fp32 rank-1 matmul broadcasts rs to all 128
     partitions (rs_col).
  6. PE rank-1 broadcast: psum[128, 512] = ones[1,128].T @ wt_row -> ScalarE
     evac to wtb_sb bf16 (softmax weights replicated on every partition).
  7. DVE weighted sum, per h-chunk k: fused tensor_tensor_reduce
     acc[:, k] = sum_s xt[k][h_local, s] * wtb_sb[h_local, s]
  8. ScalarE scales acc by rs_col -> out_sb [128, 8] fp32 -> DMA out.

PE ops for batch b-1's tail (steps 5/6) are emitted after batch b's GEMM1 so
the in-order PE never stalls on the softmax chain. bf16 operands / fp32
accumulation end-to-end rel err ~7e-3.
"""

import numpy as np

N_CORES = 8
B_FULL = 64
B_LOC = B_FULL // N_CORES  # 8 batches per core
S = 512
H = 1024
A = 256
P = 128
NK = H // P      # 8 h-chunks
NA = A // P      # 2 a-chunks

_CACHE = {}


def _build():
    import concourse.bacc as bacc
    import concourse.mybir as mybir
    import concourse.tile as tile

    F32 = mybir.dt.float32
    BF16 = mybir.dt.bfloat16
    AF = mybir.ActivationFunctionType
    ALU = mybir.AluOpType

    nc = bacc.Bacc("TRN2", target_bir_lowering=False, debug=False)

    xt_d = nc.dram_tensor("xt", [B_LOC, NK, P, S], BF16, kind="ExternalInput").ap()
    wb_d = nc.dram_tensor("wb", [H, A], BF16, kind="ExternalInput").ap()
    u_d = nc.dram_tensor("u2", [P, NA], BF16, kind="ExternalInput").ap()
    out_d = nc.dram_tensor("out", [B_LOC, H], F32, kind="ExternalOutput").ap()

    with tile.TileContext(nc) as tc:
        with (
            tc.tile_pool(name="const", bufs=1) as cpool,
            tc.tile_pool(name="xt", bufs=4) as xtpool,
            tc.tile_pool(name="ut", bufs=4) as utpool,
            tc.tile_pool(name="sm", bufs=3) as smpool,
            tc.tile_pool(name="wtb", bufs=2) as wtbpool,
            tc.tile_pool(name="scr", bufs=2) as scrpool,
            tc.tile_pool(name="o", bufs=3) as opool,
            tc.tile_pool(name="p_u", bufs=3, space="PSUM") as p_u_pool,
            tc.tile_pool(name="p_wtb", bufs=2, space="PSUM") as p_wtb_pool,
            tc.tile_pool(name="p_small", bufs=3, space="PSUM") as p_small_pool,
        ):
            # ---- constants (loaded once) ----
            wb_sb = cpool.tile([P, NK * A], BF16)  # [h_local, (k a)]
            nc.sync.dma_start(
                wb_sb[:].rearrange("p (k a) -> p k a", k=NK),
                wb_d.rearrange("(k p) a -> p k a", p=P),
            )
            u_sb = cpool.tile([P, NA], BF16)  # [a_local, a_chunk]
            nc.scalar.dma_start(u_sb[:], u_d[:])
            ones_bf = cpool.tile([1, P], BF16)
            nc.gpsimd.memset(ones_bf[:], 1.0)
            ones_f32 = cpool.tile([1, P], F32)
            nc.gpsimd.memset(ones_f32[:], 1.0)

            # PE warm-up overlapping the first xt DMA (HAM un-throttle)
            p_warm = p_u_pool.tile([P, S], F32, tag="p_u")
            for i in range(7):
                nc.tensor.matmul(
                    p_warm[:], wb_sb[:, 0:P], wb_sb[:, 0:S],
                    start=(i == 0), stop=(i == 6),
                )

            def emit_tail(xt_all_, wt_row_, rs_, b_):
                # ---- 6. PE rank-1 broadcast of wt_row to 128 partitions,
                # and rs to a [128,1] column; ScalarE evacuations ----
                p_wtb = p_wtb_pool.tile([P, S], F32, tag="p_wtb")
                nc.tensor.matmul(
                    p_wtb[:], ones_bf[:], wt_row_[:], start=True, stop=True
                )
                p_rs = p_small_pool.tile([P, 1], F32, tag="p_small")
                nc.tensor.matmul(
                    p_rs[:], ones_f32[:], rs_[:], start=True, stop=True
                )
                wtb_sb = wtbpool.tile([P, S], BF16, tag="wtb")
                nc.scalar.copy(wtb_sb[:], p_wtb[:])
                rs_col = smpool.tile([P, 1], F32, tag="rs_col")
                nc.scalar.copy(rs_col[:], p_rs[:])

                # ---- 7. DVE weighted sum: acc[:, k] = sum_s xt_k * wtb ----
                acc = opool.tile([P, NK], F32, tag="acc")
                for k in range(NK):
                    scr = scrpool.tile([P, S], BF16, tag="scr")
                    nc.vector.scalar_tensor_tensor(
                        scr[:],
                        xt_all_[:, k * S : (k + 1) * S],
                        1.0,
                        wtb_sb[:],
                        op0=ALU.mult,
                        op1=ALU.mult,
                        accum_out=acc[:, k : k + 1],
                    )

                return acc, rs_col

            def emit_out(acc_, rs_col_, b_):
                # ---- 8. normalize by rs and store. Emitted one batch after
                # emit_tail so the ScalarE never stalls waiting on the DVE
                # weighted sum; the out DMA rides the idle SWDGE queue. ----
                out_sb = opool.tile([P, NK], F32, tag="out_sb")
                nc.scalar.activation(
                    out_sb[:], acc_[:], AF.Copy, scale=rs_col_[:]
                )
                nc.gpsimd.dma_start(
                    out_d[b_].rearrange("(k p) -> p k", p=P), out_sb[:]
                )

            prev = None
            prev2 = None
            for b in range(B_LOC):
                # ---- 1. load xt[b] tiles, split across two HWDGE queues ----
                xt_all = xtpool.tile([P, NK * S], BF16, tag="xt")
                nq = 4 if b == 0 else 2  # finer slices for the pipeline head
                kq = NK // 2 // nq
                for q in range(nq):
                    nc.sync.dma_start(
                        xt_all[:, q * kq * S : (q + 1) * kq * S].rearrange(
                            "p (k s) -> p k s", k=kq
                        ),
                        xt_d[b, q * kq : (q + 1) * kq].rearrange("k p s -> p k s"),
                    )
                    off = NK // 2
                    nc.scalar.dma_start(
                        xt_all[:, (off + q * kq) * S : (off + (q + 1) * kq) * S]
                        .rearrange("p (k s) -> p k s", k=kq),
                        xt_d[b, off + q * kq : off + (q + 1) * kq].rearrange(
                            "k p s -> p k s"
                        ),
                    )
                xt_tiles = [xt_all[:, k * S : (k + 1) * S] for k in range(NK)]

                # ---- 2. GEMM1 + tanh -> u_t^T [a_local, s] ----
                ut_tiles = []
                for a in range(NA):
                    p_u = p_u_pool.tile([P, S], F32, tag="p_u")
                    for k in range(NK):
                        nc.tensor.matmul(
                            p_u[:],
                            wb_sb[:, k * A + a * P : k * A + (a + 1) * P],
                            xt_tiles[k],
                            start=(k == 0),
                            stop=(k == NK - 1),
                        )
                    ut_sb = utpool.tile([P, S], BF16, tag="ut")
                    nc.scalar.activation(ut_sb[:], p_u[:], AF.Tanh)
                    ut_tiles.append(ut_sb)

                # ---- tails of previous batches (PE work lands after this
                # batch's GEMM1, so the softmax chain latency is hidden) ----
                if prev2 is not None:
                    emit_out(*prev2)
                prev2 = None
                if prev is not None:
                    acc_rs = emit_tail(*prev)
                    prev2 = (*acc_rs, prev[3])

                # ---- 3. logit row: psum[1, s] += u[a].T @ ut[a] ----
                p_l = p_small_pool.tile([1, S], F32, tag="p_small")
                for a in range(NA):
                    nc.tensor.matmul(
                        p_l[:],
                        u_sb[:, a : a + 1],
                        ut_tiles[a][:],
                        start=(a == 0),
                        stop=(a == NA - 1),
                    )

                # ---- 4. exp (+ fused softmax sum) and 5. reciprocal ----
                wt_row = smpool.tile([1, S], BF16, tag="wt_row")
                ssum = smpool.tile([1, 1], F32, tag="ssum")
                nc.scalar.activation(
                    wt_row[:], p_l[:], AF.Exp, accum_out=ssum[:]
                )
                rs = smpool.tile([1, 1], F32, tag="rs")
                nc.vector.reciprocal(rs[:], ssum[:])

                prev = (xt_all, wt_row, rs, b)

            if prev2 is not None:
                emit_out(*prev2)
            acc_rs = emit_tail(*prev)
            emit_out(*acc_rs, prev[3])

    nc.compile()
    return nc


def get_nc():
    if "nc" not in _CACHE:
        _CACHE["nc"] = _build()
    return _CACHE["nc"]


def make_in_maps(inputs, attention_w, attention_u, attention_b):
    import ml_dtypes

    bf16 = ml_dtypes.bfloat16
    x = np.asarray(inputs, dtype=np.float32).astype(bf16)
    # block-transposed layout: xt[b, k, h_local, s] = x[b, s, k*128 + h_local]
    xt = np.ascontiguousarray(
        x.reshape(B_FULL, S, NK, P).transpose(0, 2, 3, 1)
    )
    w = np.asarray(attention_w, dtype=np.float32)
    u = np.asarray(attention_u, dtype=np.float32)
    b = np.asarray(attention_b, dtype=np.float32)
    wb = np.ascontiguousarray(w + b[None, :]).astype(bf16)
    u2 = np.ascontiguousarray(
        u[:, 0].reshape(NA, P).T.astype(bf16)
    )  # [a_local, a_chunk]
    in_maps = []
    for c in range(N_CORES):
        in_maps.append(
            {
                "xt": xt[c * B_LOC : (c + 1) * B_LOC],
                "wb": wb,
                "u2": u2,
            }
        )
    return in_maps


def kernel(inputs, attention_w, attention_u, attention_b):
    from concourse.bass_utils import run_bass_kernel_spmd

    nc = get_nc()
    in_maps = make_in_maps(inputs, attention_w, attention_u, attention_b)
    res = run_bass_kernel_spmd(nc, in_maps, list(range(N_CORES)))
    out = np.concatenate(
        [res.results[c]["out"] for c in range(N_CORES)], axis=0
    ).astype(np.float32)
    return out
